# revision 22
# baseline (speedup 1.0000x reference)
"""Pairwise cosine similarity on 8 Trainium2 NeuronCores.

Computes sim[n, m] = <x_n, y_m> / max(||x_n|| * ||y_m||, eps) for
input1 [8192, 128], input2 [8192, 128] -> out [8192, 8192] fp32.

Sharding: input1 rows are split 8 ways (data parallel, 1024 rows/core);
input2 is replicated. The host also passes each operand pre-transposed
(pure fp32 layout change, bit-exact) so DRAM loads are contiguous
16KB-per-row segments instead of 512B row gathers, and the device does
no transposes at all. Each core computes one [1024, 8192] bf16 output
stripe; the host concatenates and upcasts.

v6 design — dense-PE dataflow. Profiling v1-v4 showed three hard
lessons: (1) the PE's HAM clock gate *re-throttles to half rate after
any ~3.4us idle window*, so the PE must never starve; (2) PSUM drains
(ACT/DVE, the only engines with PSUM ports) are the throughput floor
(~43us for 64Kelem/partition); (3) DMA is packet-limited, so loads must
be big contiguous segments and stores full DRAM rows.

- PE stream: ~16 warm-up matmuls bridge the initial load phase, then
  y-norm matmuls (ones-matrix lhsT computes column sums of yT^2 -- a
  partition-axis reduction on the PE; every PSUM partition gets the
  same norms row, which makes the later column rescale elementwise),
  then the 128 main bf16 matmuls, all back-to-back.
- Norms: ||y||^2 rows drain through ACT with a fused Sqrt; DVE
  reciprocal gives 1/||y||; GPSIMD (no PSUM port, otherwise idle)
  squares yT and applies the column rescale with bf16 cast.
- x-side: 1/||x|| is computed from a small natural-layout copy of the
  x shard (per-partition stats) and folded into the PSUM drains for
  free (ACT activation scale= / DVE tensor_scalar_mul).
- Main drains alternate ACT/DVE so both PSUM ports run concurrently.
- 2-chunk-lookahead software pipeline keeps every FIFO fed without
  head-of-line blocking: loads 2 ahead on the SP ring, squares 1 ahead
  at chunk start, norm-matmuls 1 ahead mid-chunk, rescale 1 ahead at
  2/3 chunk.

Accuracy: bf16 operand + output rounding gives worst-case ~5e-3
relative-to-absmax error, well under the 2e-2 gate. The eps clamp
(1e-8) never binds: row norms are ~sqrt(128).
"""

import numpy as np

import concourse.bass as bass
import concourse.tile as tile
from concourse import bacc, mybir
from concourse.bass_utils import run_bass_kernel_spmd

N_CORES = 8
D = 128          # feature dim == partition count
P = 128          # SBUF partitions
NT = 512         # matmul moving free dim (one fp32 PSUM bank)
MMCOLS = 1024    # PSUM matmul tile columns (2 banks, 2 matmuls, 1 drain)
WARMUP = 7       # HAM warm-up matmuls bridging the first chunk load

F32 = mybir.dt.float32
BF16 = mybir.dt.bfloat16
ACTF = mybir.ActivationFunctionType


def build_nc(rows_per_core: int, corpus_rows: int) -> bass.Bass:
    # Bacc (not raw Bass): its compile() pipeline splits multi-sem waits
    # into event-semaphore instructions where a single-wait ISA slot needs
    # more than one predecessor.
    nc = bacc.Bacc(None)

    x = nc.dram_tensor("x", [rows_per_core, D], F32, kind="ExternalInput")
    xt = nc.dram_tensor("xt", [D, rows_per_core], F32, kind="ExternalInput")
    yt = nc.dram_tensor("yt", [D, corpus_rows], F32, kind="ExternalInput")
    out = nc.dram_tensor(
        "out", [rows_per_core, corpus_rows], BF16, kind="ExternalOutput"
    )

    nbx = rows_per_core // P         # x row-blocks (8)

    # Two small ramp chunks: chunk 0's prep chain gates the first main
    # matmul, so halving it starts the drain/store stream ~5us earlier.
    if corpus_rows == 8192:
        chunk_cols = [1024, 1024, 2048, 2048, 2048]
    else:
        chunk_cols = [2048] * (corpus_rows // 2048)
    assert sum(chunk_cols) == corpus_rows
    nch = len(chunk_cols)
    chunk_starts = []
    s = 0
    for cols in chunk_cols:
        chunk_starts.append(s)
        s += cols

    with tile.TileContext(nc) as tc:
        with (
            tc.tile_pool(name="const", bufs=1) as constp,
            tc.tile_pool(name="persist", bufs=1) as persist,
            tc.tile_pool(name="ld", bufs=3) as ldp,
            tc.tile_pool(name="yt", bufs=3) as ytp,
            tc.tile_pool(name="sq", bufs=2) as sqp,
            tc.tile_pool(name="inv", bufs=2) as invp,
            tc.tile_pool(name="obuf", bufs=4) as obufp,
            tc.tile_pool(name="mm", bufs=4, space=bass.MemorySpace.PSUM) as mpsum,
        ):
            wt = constp.tile([P, NT], BF16)
            nc.gpsimd.memset(wt[:], 0.0)
            ones = constp.tile([P, P], BF16)
            nc.gpsimd.memset(ones[:], 1.0)

            # PE warm-up: HAM opens the clock gate (1.2 -> 2.4 GHz) after
            # ~3.4us of sustained activity, and *re*-throttles after any
            # idle window, so these dummies are sized to end right as the
            # first norm matmuls become ready.
            wps = mpsum.tile([P, MMCOLS], F32, tag="ps")
            for _ in range(WARMUP):
                nc.tensor.matmul(wps[:, :NT], wt[:, :P], wt[:], start=True, stop=True)

            # ---- y chunk prep stages (declared before x prep so the first
            # y loads lead the SP ring: they gate the whole pipeline) ----

            ytf = {}     # chunk -> raw fp32 yT columns
            sqy = {}
            invy = {}
            yTc = {}     # chunk -> normalized bf16 yT columns

            def prep_load(c):
                cols = chunk_cols[c]
                t = ldp.tile([P, 2048], F32, tag="ld")
                nc.sync.dma_start(
                    out=t[:, :cols],
                    in_=yt[:, chunk_starts[c] : chunk_starts[c] + cols],
                )
                ytf[c] = t

            def prep_square(c):
                # The two prologue chunks run on the still-idle DVE/ACT
                # (GPSIMD's ~1.5 ns/elem would serialize the pipeline
                # ramp); steady-state chunks go to GPSIMD, otherwise free.
                cols = chunk_cols[c]
                t = sqp.tile([P, 2048], BF16, tag="sq")
                if c == 0:
                    nc.vector.tensor_mul(t[:, :cols], ytf[c][:, :cols], ytf[c][:, :cols])
                elif c == 1:
                    nc.scalar.square(t[:, :cols], ytf[c][:, :cols])
                else:
                    nc.gpsimd.tensor_mul(t[:, :cols], ytf[c][:, :cols], ytf[c][:, :cols])
                sqy[c] = t

            def prep_norm(c):
                # Column sums of yT^2 via ones-matmul (partition reduce on
                # PE), 1/sqrt fused straight into the ACT drain.
                cols = chunk_cols[c]
                inv = invp.tile([P, 2048], F32, tag="inv")
                for h0 in range(0, cols, MMCOLS):
                    hcols = min(MMCOLS, cols - h0)
                    ps = mpsum.tile([P, MMCOLS], F32)
                    for j in range(h0, h0 + hcols, NT):
                        nc.tensor.matmul(
                            ps[:, j - h0 : j - h0 + NT],
                            ones[:],
                            sqy[c][:, j : j + NT],
                            start=True,
                            stop=True,
                        )
                    nc.scalar.activation(
                        inv[:, h0 : h0 + hcols],
                        ps[:, :hcols],
                        ACTF.Abs_reciprocal_sqrt,
                    )
                invy[c] = inv

            def prep_scale(c):
                cols = chunk_cols[c]
                t = ytp.tile([P, 2048], BF16, tag="yTc")
                eng = nc.vector if c < 2 else nc.gpsimd
                eng.tensor_mul(t[:, :cols], ytf[c][:, :cols], invy[c][:, :cols])
                yTc[c] = t

            # Pipeline prologue: first y loads lead the SP ring, then the
            # x loads; chunk 0 fully prepped, chunk 1 loading.
            prep_load(0)
            if nch > 1:
                prep_load(1)

            # ---- x prep: loads + stats (per-partition 1/||x||) ----
            xtf = ldp.tile([P, rows_per_core], F32, tag="xtf")
            nc.sync.dma_start(out=xtf[:], in_=xt[:])
            xn = ldp.tile([P, nbx, D], F32, tag="xn")
            nc.sync.dma_start(
                out=xn[:], in_=x[:].rearrange("(b p) d -> p b d", p=P)
            )
            xT = persist.tile([P, rows_per_core], BF16)
            nc.gpsimd.tensor_copy(xT[:], xtf[:])
            sqx = sqp.tile([P, nbx, D], F32, tag="sqx")
            nc.gpsimd.tensor_mul(sqx[:], xn[:], xn[:])
            ssx = persist.tile([P, nbx], F32)
            nc.vector.reduce_sum(ssx[:], sqx[:], axis=mybir.AxisListType.X)
            # 1/||x|| in one ACT op. (DVE InstReciprocal measures ~5.7
            # ns/elem -- never run it on big tiles; the rsqrt ACT table is
            # plenty accurate for the 2e-2 gate.)
            invx = persist.tile([P, nbx], F32)
            nc.scalar.activation(invx[:], ssx[:], ACTF.Abs_reciprocal_sqrt)

            # Chunks 0 AND 1 fully prepped before the first main matmul:
            # a 2048-chunk's prep chain (~11us) is longer than its main
            # phase (~9.4us), so 1-deep lookahead stalls the PE at every
            # chunk boundary (HAM rethrottles on each stall).
            prep_square(0)
            prep_norm(0)
            prep_scale(0)
            if nch > 1:
                prep_square(1)
                prep_norm(1)
                prep_scale(1)

            copy_rr = 0
            for c in range(nch):
                cols = chunk_cols[c]
                col0 = chunk_starts[c]
                if c + 2 < nch:
                    prep_load(c + 2)
                for i in range(nbx):
                    if i == 2 and c + 2 < nch:
                        prep_square(c + 2)
                    if i == 4 and c + 2 < nch:
                        prep_norm(c + 2)
                    if i == 6 and c + 2 < nch:
                        prep_scale(c + 2)
                    lhs = xT[:, i * P : (i + 1) * P]
                    ob = obufp.tile([P, 2048], BF16, tag="ob")
                    for h0 in range(0, cols, MMCOLS):
                        hcols = min(MMCOLS, cols - h0)
                        ps = mpsum.tile([P, MMCOLS], F32)
                        for j in range(h0, h0 + hcols, NT):
                            nc.tensor.matmul(
                                ps[:, j - h0 : j - h0 + NT],
                                lhs,
                                yTc[c][:, j : j + NT],
                                start=True,
                                stop=True,
                            )
                        dst = ob[:, h0 : h0 + hcols]
                        # PSUM->SBUF drain, fp32->bf16 cast, with the
                        # 1/||x_row|| scale folded in for free. Measured
                        # ~1.17 (ACT) vs ~1.27 (DVE) ns/elem: split 1:1.
                        if copy_rr % 2 == 0:
                            nc.scalar.activation(
                                dst, ps[:, :hcols], ACTF.Copy,
                                scale=invx[:, i : i + 1],
                            )
                        else:
                            nc.vector.tensor_scalar_mul(
                                dst, ps[:, :hcols], invx[:, i : i + 1]
                            )
                        copy_rr += 1
                    nc.sync.dma_start(
                        out=out[i * P : (i + 1) * P, col0 : col0 + cols],
                        in_=ob[:, :cols],
                    )
                ytf.pop(c)
                sqy.pop(c, None)
                invy.pop(c, None)

    nc.finalize()  # runs Bacc.compile(): reg alloc + event-sem wait splitting
    return nc


_NC_CACHE: dict[tuple[int, int], bass.Bass] = {}


def run_spmd(input1: np.ndarray, input2: np.ndarray, **kwargs):
    """Shard, run on 8 cores, gather. Returns (output, BassKernelResults)."""
    input1 = np.ascontiguousarray(np.asarray(input1, dtype=np.float32))
    input2 = np.ascontiguousarray(np.asarray(input2, dtype=np.float32))
    n, d = input1.shape
    m, d2 = input2.shape
    assert d == D and d2 == D and n % N_CORES == 0
    rows = n // N_CORES

    key = (rows, m)
    if key not in _NC_CACHE:
        _NC_CACHE[key] = build_nc(rows, m)
    nc = _NC_CACHE[key]

    yt_full = np.ascontiguousarray(input2.T)
    in_maps = []
    for c in range(N_CORES):
        shard = input1[c * rows : (c + 1) * rows]
        in_maps.append(
            {
                "x": np.ascontiguousarray(shard),
                "xt": np.ascontiguousarray(shard.T),
                "yt": yt_full,
            }
        )
    res = run_bass_kernel_spmd(nc, in_maps, core_ids=list(range(N_CORES)), **kwargs)
    stripes = [
        np.asarray(res.results[c]["out"]).astype(np.float32) for c in range(N_CORES)
    ]
    return np.concatenate(stripes, axis=0), res


def kernel(input1: np.ndarray, input2: np.ndarray) -> np.ndarray:
    return run_spmd(input1, input2)[0]


# revision 25
# speedup vs baseline: 1.1331x; 1.1331x over previous
"""Pairwise cosine similarity on 8 Trainium2 NeuronCores.

Computes sim[n, m] = <x_n, y_m> / max(||x_n|| * ||y_m||, eps) for
input1 [8192, 128], input2 [8192, 128] -> out [8192, 8192] fp32.

Sharding: input1 rows are split 8 ways (data parallel, 1024 rows/core);
input2 is replicated. The host also passes each operand pre-transposed
(pure fp32 layout change, bit-exact) so DRAM loads are contiguous
16KB-per-row segments instead of 512B row gathers, and the device does
no transposes at all. Each core computes one [1024, 8192] bf16 output
stripe; the host concatenates and upcasts.

v6 design — dense-PE dataflow. Profiling v1-v4 showed three hard
lessons: (1) the PE's HAM clock gate *re-throttles to half rate after
any ~3.4us idle window*, so the PE must never starve; (2) PSUM drains
(ACT/DVE, the only engines with PSUM ports) are the throughput floor
(~43us for 64Kelem/partition); (3) DMA is packet-limited, so loads must
be big contiguous segments and stores full DRAM rows.

- PE stream: ~16 warm-up matmuls bridge the initial load phase, then
  y-norm matmuls (ones-matrix lhsT computes column sums of yT^2 -- a
  partition-axis reduction on the PE; every PSUM partition gets the
  same norms row, which makes the later column rescale elementwise),
  then the 128 main bf16 matmuls, all back-to-back.
- Norms: ||y||^2 rows drain through ACT with a fused Sqrt; DVE
  reciprocal gives 1/||y||; GPSIMD (no PSUM port, otherwise idle)
  squares yT and applies the column rescale with bf16 cast.
- x-side: 1/||x|| is computed from a small natural-layout copy of the
  x shard (per-partition stats) and folded into the PSUM drains for
  free (ACT activation scale= / DVE tensor_scalar_mul).
- Main drains alternate ACT/DVE so both PSUM ports run concurrently.
- 2-chunk-lookahead software pipeline keeps every FIFO fed without
  head-of-line blocking: loads 2 ahead on the SP ring, squares 1 ahead
  at chunk start, norm-matmuls 1 ahead mid-chunk, rescale 1 ahead at
  2/3 chunk.

Accuracy: bf16 operand + output rounding gives worst-case ~5e-3
relative-to-absmax error, well under the 2e-2 gate. The eps clamp
(1e-8) never binds: row norms are ~sqrt(128).
"""

import numpy as np

import concourse.bass as bass
import concourse.tile as tile
from concourse import bacc, mybir
from concourse.bass_utils import run_bass_kernel_spmd

N_CORES = 8
D = 128          # feature dim == partition count
P = 128          # SBUF partitions
NT = 512         # matmul moving free dim (one fp32 PSUM bank)
MMCOLS = 1024    # PSUM matmul tile columns (2 banks, 2 matmuls, 1 drain)
WARMUP = 18      # HAM warm-up matmuls bridging the prologue load+square
POSTWARM = 6     # dummies after the norm matmuls bridging rsqrt+scale

F32 = mybir.dt.float32
BF16 = mybir.dt.bfloat16
ACTF = mybir.ActivationFunctionType


def build_nc(rows_per_core: int, corpus_rows: int) -> bass.Bass:
    # Bacc (not raw Bass): its compile() pipeline splits multi-sem waits
    # into event-semaphore instructions where a single-wait ISA slot needs
    # more than one predecessor.
    nc = bacc.Bacc(None)

    x = nc.dram_tensor("x", [rows_per_core, D], F32, kind="ExternalInput")
    xt = nc.dram_tensor("xt", [D, rows_per_core], F32, kind="ExternalInput")
    yt = nc.dram_tensor("yt", [D, corpus_rows], F32, kind="ExternalInput")
    out = nc.dram_tensor(
        "out", [rows_per_core, corpus_rows], BF16, kind="ExternalOutput"
    )

    nbx = rows_per_core // P         # x row-blocks (8)

    chunk_cols = [2048] * (corpus_rows // 2048)
    assert sum(chunk_cols) == corpus_rows
    nch = len(chunk_cols)
    chunk_starts = []
    s = 0
    for cols in chunk_cols:
        chunk_starts.append(s)
        s += cols

    with tile.TileContext(nc) as tc:
        with (
            tc.tile_pool(name="const", bufs=1) as constp,
            tc.tile_pool(name="persist", bufs=1) as persist,
            tc.tile_pool(name="ld", bufs=3) as ldp,
            tc.tile_pool(name="yt", bufs=3) as ytp,
            tc.tile_pool(name="sq", bufs=2) as sqp,
            tc.tile_pool(name="inv", bufs=2) as invp,
            tc.tile_pool(name="obuf", bufs=4) as obufp,
            tc.tile_pool(name="mm", bufs=4, space=bass.MemorySpace.PSUM) as mpsum,
        ):
            wt = constp.tile([P, NT], BF16)
            nc.gpsimd.memset(wt[:], 0.0)
            ones = constp.tile([P, P], BF16)
            nc.gpsimd.memset(ones[:], 1.0)

            # PE warm-up: HAM opens the clock gate (1.2 -> 2.4 GHz) after
            # ~3.4us of sustained activity, and *re*-throttles after any
            # idle window, so these dummies are sized to end right as the
            # first norm matmuls become ready.
            wps = mpsum.tile([P, MMCOLS], F32, tag="ps")
            for _ in range(WARMUP):
                nc.tensor.matmul(wps[:, :NT], wt[:, :P], wt[:], start=True, stop=True)

            # ---- y chunk prep stages (declared before x prep so the first
            # y loads lead the SP ring: they gate the whole pipeline) ----

            ytf = {}     # chunk -> raw fp32 yT columns
            sqy = {}
            invy = {}
            yTc = {}     # chunk -> normalized bf16 yT columns

            def prep_load(c):
                cols = chunk_cols[c]
                t = ldp.tile([P, 2048], F32, tag="ld")
                nc.sync.dma_start(
                    out=t[:, :cols],
                    in_=yt[:, chunk_starts[c] : chunk_starts[c] + cols],
                )
                ytf[c] = t

            def prep_square(c):
                # The two prologue chunks run on the still-idle DVE/ACT
                # (GPSIMD's ~1.5 ns/elem would serialize the pipeline
                # ramp); steady-state chunks go to GPSIMD, otherwise free.
                cols = chunk_cols[c]
                t = sqp.tile([P, 2048], BF16, tag="sq")
                if c == 0:
                    nc.vector.tensor_mul(t[:, :cols], ytf[c][:, :cols], ytf[c][:, :cols])
                elif c == 1:
                    nc.scalar.square(t[:, :cols], ytf[c][:, :cols])
                else:
                    nc.gpsimd.tensor_mul(t[:, :cols], ytf[c][:, :cols], ytf[c][:, :cols])
                sqy[c] = t

            def prep_norm(c):
                # Column sums of yT^2 via ones-matmul (partition reduce on
                # PE), 1/sqrt fused straight into the ACT drain.
                cols = chunk_cols[c]
                inv = invp.tile([P, 2048], F32, tag="inv")
                for h0 in range(0, cols, MMCOLS):
                    hcols = min(MMCOLS, cols - h0)
                    ps = mpsum.tile([P, MMCOLS], F32)
                    for j in range(h0, h0 + hcols, NT):
                        nc.tensor.matmul(
                            ps[:, j - h0 : j - h0 + NT],
                            ones[:],
                            sqy[c][:, j : j + NT],
                            start=True,
                            stop=True,
                        )
                    nc.scalar.activation(
                        inv[:, h0 : h0 + hcols],
                        ps[:, :hcols],
                        ACTF.Abs_reciprocal_sqrt,
                    )
                invy[c] = inv

            def prep_scale(c):
                cols = chunk_cols[c]
                t = ytp.tile([P, 2048], BF16, tag="yTc")
                eng = nc.vector if c < 2 else nc.gpsimd
                eng.tensor_mul(t[:, :cols], ytf[c][:, :cols], invy[c][:, :cols])
                yTc[c] = t

            # Pipeline prologue: first y loads lead the SP ring, then the
            # x loads; chunk 0 fully prepped, chunk 1 loading.
            prep_load(0)
            if nch > 1:
                prep_load(1)

            # ---- x prep: loads + stats (per-partition 1/||x||) ----
            xtf = ldp.tile([P, rows_per_core], F32, tag="xtf")
            nc.sync.dma_start(out=xtf[:], in_=xt[:])
            xn = ldp.tile([P, nbx, D], F32, tag="xn")
            nc.sync.dma_start(
                out=xn[:], in_=x[:].rearrange("(b p) d -> p b d", p=P)
            )
            xT = persist.tile([P, rows_per_core], BF16)
            nc.gpsimd.tensor_copy(xT[:], xtf[:])
            sqx = sqp.tile([P, nbx, D], F32, tag="sqx")
            nc.gpsimd.tensor_mul(sqx[:], xn[:], xn[:])
            ssx = persist.tile([P, nbx], F32)
            nc.vector.reduce_sum(ssx[:], sqx[:], axis=mybir.AxisListType.X)
            # 1/||x|| in one ACT op. (DVE InstReciprocal measures ~5.7
            # ns/elem -- never run it on big tiles; the rsqrt ACT table is
            # plenty accurate for the 2e-2 gate.)
            invx = persist.tile([P, nbx], F32)
            nc.scalar.activation(invx[:], ssx[:], ACTF.Abs_reciprocal_sqrt)

            # Chunks 0 AND 1 fully prepped before the first main matmul:
            # a 2048-chunk's prep chain (~11us) is longer than its main
            # phase (~9.4us), so 1-deep lookahead stalls the PE at every
            # chunk boundary (HAM rethrottles on each stall).
            prep_square(0)
            prep_norm(0)
            prep_scale(0)
            if nch > 1:
                prep_square(1)
                prep_norm(1)
                prep_scale(1)
            # Keep the PE busy across the rsqrt->scale latency before the
            # first main matmul: any >=3.4us PE idle rethrottles HAM.
            for _ in range(POSTWARM):
                nc.tensor.matmul(wps[:, :NT], wt[:, :P], wt[:], start=True, stop=True)

            copy_rr = 0
            for c in range(nch):
                cols = chunk_cols[c]
                col0 = chunk_starts[c]
                if c + 2 < nch:
                    prep_load(c + 2)
                for i in range(nbx):
                    if i == 2 and c + 2 < nch:
                        prep_square(c + 2)
                    if i == 4 and c + 2 < nch:
                        prep_norm(c + 2)
                    if i == 6 and c + 2 < nch:
                        prep_scale(c + 2)
                    lhs = xT[:, i * P : (i + 1) * P]
                    ob = obufp.tile([P, 2048], BF16, tag="ob")
                    for h0 in range(0, cols, MMCOLS):
                        hcols = min(MMCOLS, cols - h0)
                        ps = mpsum.tile([P, MMCOLS], F32)
                        for j in range(h0, h0 + hcols, NT):
                            nc.tensor.matmul(
                                ps[:, j - h0 : j - h0 + NT],
                                lhs,
                                yTc[c][:, j : j + NT],
                                start=True,
                                stop=True,
                            )
                        dst = ob[:, h0 : h0 + hcols]
                        # PSUM->SBUF drain, fp32->bf16 cast, with the
                        # 1/||x_row|| scale folded in for free. Measured
                        # ~1.17 (ACT) vs ~1.27 (DVE) ns/elem: split 1:1.
                        if copy_rr % 2 == 0:
                            nc.scalar.activation(
                                dst, ps[:, :hcols], ACTF.Copy,
                                scale=invx[:, i : i + 1],
                            )
                        else:
                            nc.vector.tensor_scalar_mul(
                                dst, ps[:, :hcols], invx[:, i : i + 1]
                            )
                        copy_rr += 1
                    nc.sync.dma_start(
                        out=out[i * P : (i + 1) * P, col0 : col0 + cols],
                        in_=ob[:, :cols],
                    )
                ytf.pop(c)
                sqy.pop(c, None)
                invy.pop(c, None)

    nc.finalize()  # runs Bacc.compile(): reg alloc + event-sem wait splitting
    return nc


_NC_CACHE: dict[tuple[int, int], bass.Bass] = {}


def run_spmd(input1: np.ndarray, input2: np.ndarray, **kwargs):
    """Shard, run on 8 cores, gather. Returns (output, BassKernelResults)."""
    input1 = np.ascontiguousarray(np.asarray(input1, dtype=np.float32))
    input2 = np.ascontiguousarray(np.asarray(input2, dtype=np.float32))
    n, d = input1.shape
    m, d2 = input2.shape
    assert d == D and d2 == D and n % N_CORES == 0
    rows = n // N_CORES

    key = (rows, m)
    if key not in _NC_CACHE:
        _NC_CACHE[key] = build_nc(rows, m)
    nc = _NC_CACHE[key]

    yt_full = np.ascontiguousarray(input2.T)
    in_maps = []
    for c in range(N_CORES):
        shard = input1[c * rows : (c + 1) * rows]
        in_maps.append(
            {
                "x": np.ascontiguousarray(shard),
                "xt": np.ascontiguousarray(shard.T),
                "yt": yt_full,
            }
        )
    res = run_bass_kernel_spmd(nc, in_maps, core_ids=list(range(N_CORES)), **kwargs)
    stripes = [
        np.asarray(res.results[c]["out"]).astype(np.float32) for c in range(N_CORES)
    ]
    return np.concatenate(stripes, axis=0), res


def kernel(input1: np.ndarray, input2: np.ndarray) -> np.ndarray:
    return run_spmd(input1, input2)[0]
